# revision 1
# baseline (speedup 1.0000x reference)
"""GraphSAGE (2-layer, DGL SAGEConv-mean) Trainium2 kernel.

Data-parallel over B (4 samples per core, 8 cores). The whole network is
algebraically collapsed into Horner chains of A^T matmuls:

  per (b,c) pair, with A=adj, deg=max(indeg,1):
    m1 = 4*X @ A00, m4 = X @ B01, m5 = X @ C01   (host, 24x24 mats)
    R1 = A^T m1; R4 = A^T m4; R5 = A^T m5
    V2s = m4 + R5/deg;  U2s = R4 + (A^T R5)/deg
    OUT0 = m1 + 4*(A^T V2s)/deg + biasN
    OUT1 = 0.25*R1 + (A^T U2s)/deg + biasN
  out[b, 2c+k] = OUTk
  A00 = Ws0^T Ws1^T, B01 = Wn0^T Ws1^T + Ws0^T Wn1^T, C01 = Wn0^T Wn1^T
  biasN[n] = b0 Ws1^T + b1 + 1[indeg>0](n) * b0 Wn1^T

Device: 6 N^2*L-unit matmuls per pair, all node-major, stationary = raw
adj tiles (bf16 exact for 0/1), accumulation in PSUM fp32. No transposes.
"""
import sys

sys.path.insert(0, "/opt/trn_rl_repo")

import numpy as np
import ml_dtypes

from concourse import bass, bacc, tile, mybir
from concourse.bass_utils import run_bass_kernel_spmd

BF16 = mybir.dt.bfloat16
F32 = mybir.dt.float32

N = 2048
L = 24
B = 32
C = 8
NCORES = 8
BSH = B // NCORES          # 4 samples per core
NPAIR = BSH * C            # 32 (b,c) pairs per core
NT = N // 128              # 16 node tiles
NG = 2                     # pair groups per core
GP = NPAIR // NG           # 16 pairs per group
GC = GP * L                # 384 moving columns per group

_CACHE = {}


def _build_bass():
    nc = bacc.Bacc(
        "TRN2", target_bir_lowering=False, debug=False, num_devices=NCORES)
    adjb = nc.declare_dram_parameter("adjb", [128, NT * N], BF16, isOutput=False)
    m1d = nc.declare_dram_parameter("m1", [NG, 128, NT * GC], BF16, isOutput=False)
    m4d = nc.declare_dram_parameter("m4", [NG, 128, NT * GC], BF16, isOutput=False)
    m5d = nc.declare_dram_parameter("m5", [NG, 128, NT * GC], BF16, isOutput=False)
    dinvd = nc.declare_dram_parameter("dinv", [128, NT], F32, isOutput=False)
    dinv4d = nc.declare_dram_parameter("dinv4", [128, NT], F32, isOutput=False)
    biasd = nc.declare_dram_parameter("biasN", [128, NT * GC], BF16, isOutput=False)
    od = nc.declare_dram_parameter("o", [NG, NT, 2, 128, GC], F32, isOutput=True)

    mult = mybir.AluOpType.mult
    add = mybir.AluOpType.add

    with tile.TileContext(nc) as tc:
        with (
            tc.tile_pool(name="cst", bufs=1) as cst,
            tc.tile_pool(name="adjp", bufs=1) as adjp,
            tc.tile_pool(name="mov", bufs=1) as mov,
            tc.tile_pool(name="wrk", bufs=1) as wrk,
            tc.tile_pool(name="otp", bufs=4) as otp,
            tc.tile_pool(name="psp", bufs=8, space="PSUM") as psp,
        ):
            adj_sb = adjp.tile([128, NT * N], BF16)
            nc.sync.dma_start(adj_sb[:], adjb[:])
            dinv_sb = cst.tile([128, NT], F32, tag="dinv")
            nc.sync.dma_start(dinv_sb[:], dinvd[:])
            dinv4_sb = cst.tile([128, NT], F32, tag="dinv4")
            nc.sync.dma_start(dinv4_sb[:], dinv4d[:])
            bias_sb = cst.tile([128, NT * GC], BF16, tag="biasN")
            nc.sync.dma_start(bias_sb[:], biasd[:])

            def astile(u, vt):
                col = u * N + vt * 128
                return adj_sb[:, col:col + 128]

            for g in range(NG):
                m1s = mov.tile([128, NT * GC], BF16, tag="m1")
                m4s = mov.tile([128, NT * GC], BF16, tag="m4")
                m5s = mov.tile([128, NT * GC], BF16, tag="m5")
                nc.sync.dma_start(m1s[:], m1d[g])
                nc.sync.dma_start(m4s[:], m4d[g])
                nc.sync.dma_start(m5s[:], m5d[g])

                R1 = wrk.tile([128, NT * GC], BF16, tag="R1")
                R4 = wrk.tile([128, NT * GC], BF16, tag="R4")
                R5 = wrk.tile([128, NT * GC], BF16, tag="R5")
                V2s = wrk.tile([128, NT * GC], BF16, tag="V2s")
                U2s = wrk.tile([128, NT * GC], BF16, tag="U2s")

                # Stage P: R1/R4/R5 = A^T {m1,m4,m5}; V2s = m4 + R5/deg
                for vt in range(NT):
                    sl = slice(vt * GC, (vt + 1) * GC)
                    dv = dinv_sb[:, vt:vt + 1]
                    for which in range(3):
                        src = (m1s, m4s, m5s)[which]
                        ps = psp.tile([128, GC], F32)
                        for u in range(NT):
                            nc.tensor.matmul(
                                ps[:], astile(u, vt), src[:, u * GC:(u + 1) * GC],
                                start=(u == 0), stop=(u == NT - 1),
                            )
                        if which == 0:
                            nc.vector.tensor_copy(R1[:, sl], ps[:])
                        elif which == 1:
                            nc.vector.tensor_copy(R4[:, sl], ps[:])
                        else:
                            nc.vector.tensor_copy(R5[:, sl], ps[:])
                            nc.vector.scalar_tensor_tensor(
                                V2s[:, sl], ps[:], dv, m4s[:, sl],
                                op0=mult, op1=add)

                # Stage U: U2s = R4 + (A^T R5)/deg
                for vt in range(NT):
                    sl = slice(vt * GC, (vt + 1) * GC)
                    ps = psp.tile([128, GC], F32)
                    for u in range(NT):
                        nc.tensor.matmul(
                            ps[:], astile(u, vt), R5[:, u * GC:(u + 1) * GC],
                            start=(u == 0), stop=(u == NT - 1))
                    nc.vector.scalar_tensor_tensor(
                        U2s[:, sl], ps[:], dinv_sb[:, vt:vt + 1], R4[:, sl],
                        op0=mult, op1=add)

                # Stage OUT0 = m1 + 4*(A^T V2s)/deg + biasN
                for vt in range(NT):
                    sl = slice(vt * GC, (vt + 1) * GC)
                    ps = psp.tile([128, GC], F32)
                    for u in range(NT):
                        nc.tensor.matmul(
                            ps[:], astile(u, vt), V2s[:, u * GC:(u + 1) * GC],
                            start=(u == 0), stop=(u == NT - 1))
                    t0 = otp.tile([128, GC], F32, tag="t0")
                    nc.vector.scalar_tensor_tensor(
                        t0[:], ps[:], dinv4_sb[:, vt:vt + 1], m1s[:, sl],
                        op0=mult, op1=add)
                    t0b = otp.tile([128, GC], F32, tag="t0b")
                    nc.vector.tensor_tensor(
                        t0b[:], t0[:], bias_sb[:, sl], op=add)
                    nc.sync.dma_start(od[g, vt, 0], t0b[:])

                # Stage OUT1 = 0.25*R1 + (A^T U2s)/deg + biasN
                for vt in range(NT):
                    sl = slice(vt * GC, (vt + 1) * GC)
                    ps = psp.tile([128, GC], F32)
                    for u in range(NT):
                        nc.tensor.matmul(
                            ps[:], astile(u, vt), U2s[:, u * GC:(u + 1) * GC],
                            start=(u == 0), stop=(u == NT - 1))
                    t1 = otp.tile([128, GC], F32, tag="t1")
                    nc.vector.scalar_tensor_tensor(
                        t1[:], ps[:], dinv_sb[:, vt:vt + 1], bias_sb[:, sl],
                        op0=mult, op1=add)
                    t1b = otp.tile([128, GC], F32, tag="t1b")
                    nc.vector.scalar_tensor_tensor(
                        t1b[:], R1[:, sl], 0.25, t1[:], op0=mult, op1=add)
                    nc.sync.dma_start(od[g, vt, 1], t1b[:])
    nc.compile()
    return nc


def _pack_moving(m):
    """[BSH, C, N, L] f32 -> [NG, 128, NT*GC] bf16 (pairs b-major)."""
    a = m.transpose(2, 0, 1, 3).reshape(NT, 128, NPAIR * L)
    a = a.reshape(NT, 128, NG, GC).transpose(2, 1, 0, 3).reshape(NG, 128, NT * GC)
    return np.ascontiguousarray(a).astype(ml_dtypes.bfloat16)


def kernel(x, adj, W_self, W_neigh, bias, _trace=False):
    x = np.asarray(x, dtype=np.float32)
    adj = np.asarray(adj, dtype=np.float32)
    W_self = np.asarray(W_self, dtype=np.float32)
    W_neigh = np.asarray(W_neigh, dtype=np.float32)
    bias = np.asarray(bias, dtype=np.float32)

    A00 = W_self[0].T @ W_self[1].T
    B01 = W_neigh[0].T @ W_self[1].T + W_self[0].T @ W_neigh[1].T
    C01 = W_neigh[0].T @ W_neigh[1].T
    indeg = adj.sum(0)
    deg = np.maximum(indeg, 1.0)
    s = (indeg >= 1).astype(np.float32)
    biasN = (bias[0] @ W_self[1].T + bias[1])[None, :] \
        + s[:, None] * (bias[0] @ W_neigh[1].T)[None, :]      # [N, L]

    adjb = np.ascontiguousarray(
        adj.reshape(NT, 128, N).transpose(1, 0, 2).reshape(128, NT * N)
    ).astype(ml_dtypes.bfloat16)
    dinv = np.ascontiguousarray((1.0 / deg).reshape(NT, 128).T).astype(np.float32)
    dinv4 = np.ascontiguousarray(4.0 * dinv)
    biasP = np.ascontiguousarray(
        np.broadcast_to(biasN.reshape(NT, 128, 1, L), (NT, 128, GP, L))
        .reshape(NT, 128, GC).transpose(1, 0, 2).reshape(128, NT * GC)
    ).astype(ml_dtypes.bfloat16)

    m1_all = 4.0 * (x @ A00)
    m4_all = x @ B01
    m5_all = x @ C01

    if "nc" not in _CACHE:
        _CACHE["nc"] = _build_bass()
    nc = _CACHE["nc"]

    in_maps = []
    for c in range(NCORES):
        sl = slice(c * BSH, (c + 1) * BSH)
        in_maps.append({
            "adjb": adjb,
            "m1": _pack_moving(m1_all[sl]),
            "m4": _pack_moving(m4_all[sl]),
            "m5": _pack_moving(m5_all[sl]),
            "dinv": dinv,
            "dinv4": dinv4,
            "biasN": biasP,
        })

    res = run_bass_kernel_spmd(
        nc, in_maps, list(range(NCORES)), trace=_trace)

    out = np.empty((B, 2 * C, N, L), dtype=np.float32)
    for c in range(NCORES):
        o = np.asarray(res.results[c]["o"], dtype=np.float32)
        # [NG, NT, 2, 128, GC] -> (g, vt, k, p, pin, l)
        a = o.reshape(NG, NT, 2, 128, GP, L)
        # pairs = g*GP + pin, b-major: b_local = pairs//C, ch = pairs%C
        a = a.transpose(0, 4, 2, 1, 3, 5).reshape(NPAIR, 2, N, L)
        a = a.reshape(BSH, C, 2, N, L).reshape(BSH, 2 * C, N, L)
        out[c * BSH:(c + 1) * BSH] = a
    if _trace:
        return out, res
    return out

